# revision 24
# baseline (speedup 1.0000x reference)
"""Trainium2 Bass kernel for a 2-layer GAT with edge-weighted aggregation.

Sharding: nodes partitioned across 8 cores by dst range; edges assigned to the
core owning their dst node (host-side sort); weight matrices replicated.
Per-edge gathers use GPSIMD dma_gather; per-dst segment reductions are
selection-matrix matmuls on the TensorEngine accumulating in PSUM; node tables
are exchanged between layers via AllGather collectives.

v2: one packed per-layer table row [feat bf16 x128 | el f32 x4 | er f32 x4 |
pad] = 512B so the GAT pass needs one full-rate src gather (feat+el) plus one
dst gather of the el/er half-row; selection matrices built with single
tensor_scalar ops (per-partition scalar operands keep the DVE fast mode);
exp(leaky_relu(x)) computed as max(exp(x), exp(0.2x)) on the Scalar engine.
"""

import sys

sys.path.insert(0, "/opt/trn_rl_repo")

import numpy as np
import ml_dtypes

import concourse.bacc as bacc
import concourse.bass as bass
import concourse.mybir as mybir
import concourse.tile as tile
from concourse.bass_utils import run_bass_kernel_spmd

BF16 = ml_dtypes.bfloat16
F32 = np.float32

# full-problem constants
N_NODES = 50000
N_EDGES = 800000
D = 128
H = 4
DH = 32
NEG_SLOPE = 0.2
NCORES = 8
PROW = 256          # packed table row: 256 bf16 = 512B


class Cfg:
    def __init__(self, n_nodes, ncores, T, CB):
        self.N = n_nodes
        self.NC = ncores
        self.NL = n_nodes // ncores          # real nodes per core
        self.NBLK = -(-self.NL // 128)       # node blocks per core
        self.NLP = self.NBLK * 128           # padded nodes per core
        self.NP = ncores * self.NLP          # padded global rows
        self.GROW = (ncores // 2) * self.NLP  # padded rows per src group
        self.T = T                           # edge tiles per (group, block)
        self.CB = CB                         # blocks per gather chunk
        assert self.NBLK % CB == 0
        self.NCHUNK = self.NBLK // CB
        self.S = 2 * self.NBLK * T * 128     # edge slots per core


def _wrap_idx(idx_flat):
    """Logical idx list [S] -> [128, S/16] int16 (16-way wrap, x8 replicas)."""
    a = np.ascontiguousarray(idx_flat.reshape(-1, 16).T.astype(np.int16))
    return np.ascontiguousarray(np.tile(a, (8, 1)))


def _prep_host(cfg, in_feat, edge_weights, src, dst, b0):
    """Build all per-core host arrays."""
    NL, NLP, NBLK, T, S = cfg.NL, cfg.NLP, cfg.NBLK, cfg.T, cfg.S
    half = cfg.N // 2
    c = dst // NL
    dl = dst - c * NL
    b = dl // 128
    g = (src >= half).astype(np.int64)
    psrc = (src // NL) * NLP + (src % NL)
    gidx_val = (psrc - g * cfg.GROW).astype(np.int64)
    assert gidx_val.min() >= 0 and gidx_val.max() < cfg.GROW < 32768

    cell = ((c * NBLK + b) * 2 + g).astype(np.int64)
    order = np.argsort(cell, kind="stable")
    cell_s = cell[order]
    counts = np.bincount(cell_s, minlength=cfg.NC * NBLK * 2)
    assert counts.max() <= T * 128, (counts.max(), T * 128)
    within = np.arange(len(cell_s), dtype=np.int64)
    seg_starts = np.r_[0, np.flatnonzero(np.diff(cell_s)) + 1]
    seg_lens = np.diff(np.r_[seg_starts, len(cell_s)])
    within -= np.repeat(seg_starts, seg_lens)

    co = c[order]
    go = g[order]
    bo = b[order]
    slot = (go * NBLK + bo) * T * 128 + within  # local slot within core

    gidx_a = np.zeros((cfg.NC, S), dtype=np.int64)
    dgidx_a = np.zeros((cfg.NC, S), dtype=np.int64)
    dstin_a = np.full((cfg.NC, S), -1.0, dtype=F32)
    wts_a = np.zeros((cfg.NC, S), dtype=F32)
    gidx_a[co, slot] = gidx_val[order]
    dgidx_a[co, slot] = dl[order]
    dstin_a[co, slot] = (dl[order] - bo * 128).astype(F32)
    wts_a[co, slot] = edge_weights[order]

    per_core = []
    for ci in range(cfg.NC):
        sl = in_feat[ci * NL:(ci + 1) * NL]
        infeat = np.zeros((NLP, D), dtype=F32)
        infeat[:NL] = sl
        infeatb0 = np.zeros((NLP, D), dtype=F32)
        infeatb0[:NL] = sl + b0[None, :]
        per_core.append(dict(
            gidx=_wrap_idx(gidx_a[ci]),
            dgidx=_wrap_idx(dgidx_a[ci]),
            dstin=np.ascontiguousarray(dstin_a[ci].reshape(-1, 128).T),
            wts=np.ascontiguousarray(wts_a[ci].reshape(-1, 128).T),
            infeat=infeat,
            infeatb0=infeatb0,
        ))

    h0tab = np.zeros((cfg.NP, D), dtype=np.float16)
    for ci in range(cfg.NC):
        h0tab[ci * NLP:ci * NLP + NL] = in_feat[ci * NL:(ci + 1) * NL].astype(
            np.float16)
    return per_core, h0tab


def _alar(al, ar):
    """[H,DH] x2 -> [128, 8] f32 selector for el/er column extraction."""
    m = np.zeros((D, 2 * H), dtype=F32)
    for h in range(H):
        m[h * DH:(h + 1) * DH, h] = al[h]
        m[h * DH:(h + 1) * DH, H + h] = ar[h]
    return m


def _build(cfg, for_sim=False, stages=5, no_cc=False):
    """Build the SPMD Bass program (identical on all cores)."""
    dt = mybir.dt
    nc = bacc.Bacc("TRN2", debug=for_sim, target_bir_lowering=not for_sim,
                   num_devices=cfg.NC, num_swdge_queues=4)
    NBLK, T, CB, S, NP, NLP = cfg.NBLK, cfg.T, cfg.CB, cfg.S, cfg.NP, cfg.NLP
    GROW = cfg.GROW
    groups = [list(range(cfg.NC))]
    is_eq = mybir.AluOpType.is_equal
    mult = mybir.AluOpType.mult
    add = mybir.AluOpType.add
    amax = mybir.AluOpType.max

    # external inputs
    h0tab = nc.dram_tensor("h0tab", [NP, D], dt.float16, kind="ExternalInput")
    infeat = nc.dram_tensor("infeat", [NLP, D], dt.float32,
                            kind="ExternalInput")
    infeatb0 = nc.dram_tensor("infeatb0", [NLP, D], dt.float32,
                              kind="ExternalInput")
    w0 = nc.dram_tensor("w0", [D, D], dt.float32, kind="ExternalInput")
    w1 = nc.dram_tensor("w1", [D, D], dt.float32, kind="ExternalInput")
    alar0 = nc.dram_tensor("alar0", [D, 2 * H], dt.float32,
                           kind="ExternalInput")
    alar1 = nc.dram_tensor("alar1", [D, 2 * H], dt.float32,
                           kind="ExternalInput")
    b1t = nc.dram_tensor("b1t", [128, D], dt.float32, kind="ExternalInput")
    iota_in = nc.dram_tensor("iota", [128, 128], dt.float16,
                             kind="ExternalInput")
    ident_in = nc.dram_tensor("ident", [128, 128], dt.float32,
                              kind="ExternalInput")
    gidx = nc.dram_tensor("gidx", [128, S // 16], dt.int16,
                          kind="ExternalInput")
    dgidx = nc.dram_tensor("dgidx", [128, S // 16], dt.int16,
                           kind="ExternalInput")
    dstin_t = nc.dram_tensor("dstin", [128, S // 128], dt.float32,
                             kind="ExternalInput")
    wts_t = nc.dram_tensor("wts", [128, S // 128], dt.float32,
                           kind="ExternalInput")
    out_t = nc.dram_tensor("out", [NLP, D], dt.float32, kind="ExternalOutput")

    with tile.TileContext(nc) as tc:
        with (
            tc.tile_pool(name="dram", bufs=1, space="DRAM") as dram,
            tc.tile_pool(name="const", bufs=1) as pc,
            tc.tile_pool(name="acc", bufs=1) as pacc,
            tc.tile_pool(name="idx", bufs=3) as pidx,
            tc.tile_pool(name="gath", bufs=2) as pg,
            tc.tile_pool(name="work", bufs=3) as pw,
            tc.tile_pool(name="sel", bufs=6) as psel,
            tc.tile_pool(name="small", bufs=4) as psm,
            tc.tile_pool(name="ps", bufs=4, space="PSUM") as pps,
            tc.tile_pool(name="pss", bufs=2, space="PSUM") as ppss,
        ):
            # internal DRAM: packed per-layer tables + h2 table
            ptab_slice = [dram.tile([NLP, PROW], dt.bfloat16,
                                    name=f"ptabsl{l}") for l in range(2)]
            ptab_full = [dram.tile([NP, PROW], dt.bfloat16,
                                   addr_space="Shared", name=f"ptabfu{l}")
                         for l in range(2)]
            h2slice = dram.tile([NLP, D], dt.float16, name="h2slice")
            h2full = dram.tile([NP, D], dt.float16, addr_space="Shared",
                               name="h2full")

            # resident SBUF
            iota_sb = pc.tile([128, 128], dt.float16, name="iota_sb")
            nc.sync.dma_start(out=iota_sb[:], in_=iota_in[:])
            ident_sb = pc.tile([128, 128], dt.float32, name="ident_sb")
            nc.sync.dma_start(out=ident_sb[:], in_=ident_in[:])
            w_sb = [pc.tile([D, D], dt.float32, name=f"w_sb{l}")
                    for l in range(2)]
            nc.sync.dma_start(out=w_sb[0][:], in_=w0[:])
            nc.sync.dma_start(out=w_sb[1][:], in_=w1[:])
            alar_sb = [pc.tile([D, 2 * H], dt.float32, name=f"alar_sb{l}")
                       for l in range(2)]
            nc.sync.dma_start(out=alar_sb[0][:], in_=alar0[:])
            nc.sync.dma_start(out=alar_sb[1][:], in_=alar1[:])
            b1_sb = pc.tile([128, D], dt.float32, name="b1_sb")
            nc.sync.dma_start(out=b1_sb[:], in_=b1t[:])
            dstin_sb = pc.tile([128, S // 128], dt.float32, name="dstin_sb")
            nc.sync.dma_start(out=dstin_sb[:], in_=dstin_t[:])
            wts_sb = pc.tile([128, S // 128], dt.float32, name="wts_sb")
            nc.sync.dma_start(out=wts_sb[:], in_=wts_t[:])
            elrnm_sb = pc.tile([128, NBLK * 8], dt.float32, name="elrnm_sb")

            acc_a = pacc.tile([128, NBLK * 132], dt.float32, name="acc_a")
            acc_b = pacc.tile([128, NBLK * 132], dt.float32, name="acc_b")

            qn = [0]

            def next_q():
                return 0

            def maybe_cc(ins_ap, outs_ap):
                if no_cc:
                    return
                nc.gpsimd.collective_compute(
                    "AllGather", mybir.AluOpType.bypass,
                    replica_groups=groups, ins=[ins_ap], outs=[outs_ap])

            def agg_pass(src_tab, dest_acc, init_from_infeat,
                         after_block=None):
                """dest_acc[:, b*132:+128] (+)= sum_e w_e * tab[src_e]."""
                for bc in range(cfg.NCHUNK):
                    for g in range(2):
                        slot0 = (g * NBLK + bc * CB) * T * 128
                        nsl = CB * T * 128
                        idxt = pidx.tile([128, nsl // 16], dt.int16,
                                         name="idxa", tag="gi")
                        nc.sync.dma_start(
                            out=idxt[:],
                            in_=gidx[:, slot0 // 16:(slot0 + nsl) // 16])
                        hg = pg.tile([128, CB * T, 128], dt.float16,
                                     name="hg", tag="fgb")
                        nc.gpsimd.dma_gather(
                            hg[:], src_tab[g * GROW:(g + 1) * GROW, :],
                            idxt[:], nsl, nsl, 128, queue_num=next_q(),
                            single_packet=False)
                        for bi in range(CB):
                            b = bc * CB + bi
                            gt0 = (g * NBLK + b) * T
                            ps = pps.tile([128, 128], dt.float32, name="psa",
                                          tag="scat")
                            for t in range(T):
                                selw = psel.tile([128, 128], dt.float16,
                                                 name="selw", tag="sel")
                                nc.vector.tensor_scalar(
                                    out=selw[:], in0=iota_sb[:],
                                    scalar1=dstin_sb[:, gt0 + t:gt0 + t + 1],
                                    scalar2=wts_sb[:, gt0 + t:gt0 + t + 1],
                                    op0=is_eq, op1=mult)
                                nc.tensor.matmul(
                                    ps[:], selw[:],
                                    hg[:, bi * T + t, :],
                                    start=(t == 0), stop=(t == T - 1))
                            dsl = dest_acc[:, b * 132:b * 132 + 128]
                            if init_from_infeat and g == 0:
                                it = psm.tile([128, 128], dt.float32,
                                              name="ift", tag="ift")
                                nc.sync.dma_start(
                                    out=it[:],
                                    in_=infeat[b * 128:(b + 1) * 128, :])
                                nc.vector.tensor_tensor(
                                    out=dsl, in0=ps[:], in1=it[:], op=add)
                            else:
                                nc.vector.tensor_tensor(
                                    out=dsl, in0=ps[:], in1=dsl, op=add)
                            if g == 1 and after_block is not None:
                                after_block(b)

            def feat_block(src_acc, layer, b):
                    hsl = src_acc[:, b * 132:b * 132 + 128]
                    tp = pps.tile([128, 128], dt.float32, name="tp",
                                  tag="scat")
                    nc.tensor.transpose(out=tp[:], in_=hsl,
                                        identity=ident_sb[:])
                    hT = psm.tile([128, 128], dt.float32, name="hT", tag="hT")
                    nc.vector.tensor_copy(out=hT[:], in_=tp[:])
                    fp = pps.tile([128, 128], dt.float32, name="fp",
                                  tag="scat")
                    nc.tensor.matmul(fp[:], w_sb[layer][:], hT[:],
                                     start=True, stop=True)
                    fT = psm.tile([128, 128], dt.float32, name="fT", tag="fT")
                    nc.vector.tensor_copy(out=fT[:], in_=fp[:])
                    ep = ppss.tile([2 * H, 128], dt.float32, name="ep",
                                   tag="ep")
                    nc.tensor.matmul(ep[:], alar_sb[layer][:], fT[:],
                                     start=True, stop=True)
                    eT = psm.tile([2 * H, 128], dt.float32, name="eT",
                                  tag="eT")
                    nc.vector.tensor_copy(out=eT[:], in_=ep[:])
                    enm = ppss.tile([128, 2 * H], dt.float32, name="enm",
                                    tag="ep")
                    nc.tensor.matmul(enm[:], eT[:],
                                     ident_sb[0:2 * H, 0:2 * H],
                                     start=True, stop=True)
                    nc.vector.tensor_copy(out=elrnm_sb[:, b * 8:(b + 1) * 8],
                                          in_=enm[:])
                    ftp = pps.tile([128, 128], dt.float32, name="ftp",
                                   tag="scat")
                    nc.tensor.transpose(out=ftp[:], in_=fT[:],
                                        identity=ident_sb[:])
                    fnm = psm.tile([128, 128], dt.bfloat16, name="fnm",
                                   tag="fnm")
                    nc.vector.tensor_copy(out=fnm[:], in_=ftp[:])
                    nc.sync.dma_start(
                        out=ptab_slice[layer][b * 128:(b + 1) * 128, 0:D],
                        in_=fnm[:])

            def feat_tail(layer):
                # el/er columns: f32 values bit-cast into the bf16 table
                nc.sync.dma_start(
                    out=ptab_slice[layer][:, D:D + 16].rearrange(
                        "(b p) c -> p b c", p=128),
                    in_=elrnm_sb[:].bitcast(dt.bfloat16).rearrange(
                        "p (b c) -> p b c", c=16))
                maybe_cc(ptab_slice[layer].opt(), ptab_full[layer].opt())

            def gat_pass(layer, dest_acc, after_block=None):
                """GAT message pass accumulating [usum | s] into dest_acc."""
                for bc in range(cfg.NCHUNK):
                    for g in range(2):
                        slot0 = (g * NBLK + bc * CB) * T * 128
                        nsl = CB * T * 128
                        idxt = pidx.tile([128, nsl // 16], dt.int16,
                                         name="idxg", tag="gi")
                        nc.sync.dma_start(
                            out=idxt[:],
                            in_=gidx[:, slot0 // 16:(slot0 + nsl) // 16])
                        didxt = pidx.tile([128, nsl // 16], dt.int16,
                                          name="idxd", tag="di")
                        nc.sync.dma_start(
                            out=didxt[:],
                            in_=dgidx[:, slot0 // 16:(slot0 + nsl) // 16])
                        fg = pg.tile([128, CB * T, PROW], dt.bfloat16,
                                     name="fg", tag="fgb")
                        nc.gpsimd.dma_gather(
                            fg[:],
                            ptab_full[layer][g * GROW:(g + 1) * GROW, :],
                            idxt[:], nsl, nsl, PROW, queue_num=next_q(),
                            single_packet=False)
                        eg2 = pg.tile([128, CB * T, 128], dt.bfloat16,
                                      name="eg2", tag="eg2")
                        nc.gpsimd.dma_gather(
                            eg2[:], ptab_slice[layer][:, D:PROW],
                            didxt[:], nsl, nsl, 128, elem_step=PROW,
                            queue_num=next_q(), single_packet=False)
                        # e = el[src] + er[dst]; ee = max(exp(e), exp(.2 e))
                        ev = pw.tile([128, CB * T * H], dt.float32,
                                     name="ev", tag="ev")
                        nc.vector.tensor_tensor(
                            out=ev[:].rearrange("p (c h) -> p c h", h=H),
                            in0=fg[:, :, D:D + 8].bitcast(dt.float32),
                            in1=eg2[:, :, 8:16].bitcast(dt.float32),
                            op=add)
                        e1 = pw.tile([128, CB * T * H], dt.float32,
                                     name="e1", tag="e1")
                        nc.scalar.activation(
                            out=e1[:], in_=ev[:],
                            func=mybir.ActivationFunctionType.Exp)
                        e2 = pw.tile([128, CB * T * H], dt.float32,
                                     name="e2", tag="e2")
                        nc.scalar.activation(
                            out=e2[:], in_=ev[:],
                            func=mybir.ActivationFunctionType.Exp,
                            scale=NEG_SLOPE)
                        eeb = pw.tile([128, CB * T * H], dt.bfloat16,
                                      name="eeb", tag="eeb")
                        nc.vector.tensor_tensor(out=eeb[:], in0=e1[:],
                                                in1=e2[:], op=amax)
                        for bi in range(CB):
                            b = bc * CB + bi
                            gt0 = (g * NBLK + b) * T
                            rsc = pw.tile([128, T * 128], dt.bfloat16,
                                          name="rsc", tag="rsc")
                            for t in range(T):
                                c = bi * T + t
                                nc.vector.tensor_tensor(
                                    out=rsc[:, t * 128:(t + 1) * 128
                                            ].rearrange(
                                        "p (h d) -> p h d", h=H),
                                    in0=fg[:, c, 0:D].rearrange(
                                        "p (h d) -> p h d", h=H),
                                    in1=eeb[:, c * H:(c + 1) * H, None
                                            ].to_broadcast([128, H, DH]),
                                    op=mult)
                            ps = pps.tile([128, 128], dt.float32, name="psg",
                                          tag="scat")
                            ps_s = ppss.tile([128, H], dt.float32,
                                             name="pss", tag="ep")
                            for t in range(T):
                                sel = psel.tile([128, 128], dt.bfloat16,
                                                name="selg", tag="sel")
                                nc.vector.tensor_scalar(
                                    out=sel[:], in0=iota_sb[:],
                                    scalar1=dstin_sb[:, gt0 + t:gt0 + t + 1],
                                    scalar2=None, op0=is_eq)
                                nc.tensor.matmul(
                                    ps[:], sel[:],
                                    rsc[:, t * 128:(t + 1) * 128],
                                    start=(t == 0), stop=(t == T - 1))
                                nc.tensor.matmul(
                                    ps_s[:], sel[:],
                                    eeb[:, (bi * T + t) * H:
                                        (bi * T + t + 1) * H],
                                    start=(t == 0), stop=(t == T - 1))
                            dsl = dest_acc[:, b * 132:b * 132 + 128]
                            dss = dest_acc[:, b * 132 + 128:b * 132 + 132]
                            if g == 0:
                                nc.vector.tensor_copy(out=dsl, in_=ps[:])
                                nc.vector.tensor_copy(out=dss, in_=ps_s[:])
                            else:
                                nc.vector.tensor_tensor(
                                    out=dsl, in0=ps[:], in1=dsl, op=add)
                                nc.vector.tensor_tensor(
                                    out=dss, in0=ps_s[:], in1=dss, op=add)
                                if after_block is not None:
                                    after_block(b)

            def fin1(b):
                ssb = psm.tile([128, H], dt.float32, name="ssb", tag="ssb")
                nc.vector.tensor_scalar_add(
                    out=ssb[:], in0=acc_a[:, b * 132 + 128:b * 132 + 132],
                    scalar1=1e-30)
                sr = psm.tile([128, H], dt.float32, name="sr", tag="sr")
                nc.vector.reciprocal(out=sr[:], in_=ssb[:])
                rst = psm.tile([128, 128], dt.float32, name="rst", tag="rst")
                nc.vector.tensor_tensor(
                    out=rst[:].rearrange("p (h d) -> p h d", h=H),
                    in0=acc_a[:, b * 132:b * 132 + 128].rearrange(
                        "p (h d) -> p h d", h=H),
                    in1=sr[:, :, None].to_broadcast([128, H, DH]),
                    op=mult)
                it = psm.tile([128, 128], dt.float32, name="ifb", tag="ift")
                nc.sync.dma_start(out=it[:],
                                  in_=infeatb0[b * 128:(b + 1) * 128, :])
                hb = psm.tile([128, 128], dt.float32, name="hb", tag="hb")
                nc.vector.tensor_tensor(out=hb[:], in0=rst[:], in1=it[:],
                                        op=add)
                nc.scalar.activation(
                    out=acc_b[:, b * 132:b * 132 + 128], in_=hb[:],
                    func=mybir.ActivationFunctionType.Relu)
                h2b = psm.tile([128, 128], dt.float16, name="h2b", tag="h2b")
                nc.vector.tensor_copy(out=h2b[:],
                                      in_=acc_b[:, b * 132:b * 132 + 128])
                nc.sync.dma_start(out=h2slice[b * 128:(b + 1) * 128, :],
                                  in_=h2b[:])

            def fin2(b):
                ssb = psm.tile([128, H], dt.float32, name="ssb2", tag="ssb")
                nc.vector.tensor_scalar_add(
                    out=ssb[:], in0=acc_a[:, b * 132 + 128:b * 132 + 132],
                    scalar1=1e-30)
                sr = psm.tile([128, H], dt.float32, name="sr2", tag="sr")
                nc.vector.reciprocal(out=sr[:], in_=ssb[:])
                rst = psm.tile([128, 128], dt.float32, name="rst2", tag="rst")
                nc.vector.tensor_tensor(
                    out=rst[:].rearrange("p (h d) -> p h d", h=H),
                    in0=acc_a[:, b * 132:b * 132 + 128].rearrange(
                        "p (h d) -> p h d", h=H),
                    in1=sr[:, :, None].to_broadcast([128, H, DH]),
                    op=mult)
                ob = psm.tile([128, 128], dt.float32, name="ob", tag="hb")
                nc.vector.tensor_tensor(out=ob[:], in0=rst[:], in1=b1_sb[:],
                                        op=add)
                nc.sync.dma_start(out=out_t[b * 128:(b + 1) * 128, :],
                                  in_=ob[:])

            # ---------------- layer 1 ----------------
            agg_pass(h0tab, acc_a, init_from_infeat=True,
                     after_block=(lambda b: feat_block(acc_a, 0, b))
                     if stages >= 2 else None)
            if stages >= 2:
                feat_tail(0)
            if stages >= 3:
                gat_pass(0, acc_a, after_block=fin1)
                maybe_cc(h2slice.opt(), h2full.opt())

            # ---------------- layer 2 ----------------
            if stages >= 4:
                agg_pass(h2full, acc_b, init_from_infeat=False,
                         after_block=lambda b: feat_block(acc_b, 1, b))
                feat_tail(1)
            if stages >= 5:
                gat_pass(1, acc_a, after_block=fin2)
            if stages < 5:
                src_acc = acc_a if stages < 4 else acc_b
                for b in range(NBLK):
                    nc.sync.dma_start(
                        out=out_t[b * 128:(b + 1) * 128, :],
                        in_=src_acc[:, b * 132:b * 132 + 128])

    nc.compile()
    return nc


def _compute_T(cfg_n, ncores, src, dst):
    NL = cfg_n // ncores
    NBLK = -(-NL // 128)
    c = dst // NL
    dl = dst - c * NL
    b = dl // 128
    g = (src >= cfg_n // 2).astype(np.int64)
    cell = (c * NBLK + b) * 2 + g
    cnt = np.bincount(cell, minlength=ncores * NBLK * 2)
    return int(-(-cnt.max() // 128))


def run_gat(n_nodes, ncores, CB, in_feat, edge_weights, W0, al0, ar0, b0,
            W1, al1, ar1, b1, src, dst, trace=False):
    """Shared entry: build, run on hardware, return full output."""
    T = _compute_T(n_nodes, ncores, src, dst)
    cfg = Cfg(n_nodes, ncores, T, CB)
    per_core, h0tab = _prep_host(cfg, in_feat, edge_weights, src, dst, b0)
    nc = _build(cfg)

    iota = np.tile(np.arange(128, dtype=F32)[None, :], (128, 1)).astype(
        np.float16)
    ident = np.eye(128, dtype=F32)
    shared = dict(
        h0tab=h0tab, w0=W0.astype(F32), w1=W1.astype(F32),
        alar0=_alar(al0, ar0), alar1=_alar(al1, ar1),
        b1t=np.tile(b1[None, :], (128, 1)).astype(F32),
        iota=np.ascontiguousarray(iota), ident=ident,
    )
    in_maps = []
    for ci in range(ncores):
        m = dict(shared)
        m.update(per_core[ci])
        in_maps.append(m)
    res = run_bass_kernel_spmd(nc, in_maps, core_ids=list(range(ncores)),
                               trace=trace)
    out = np.concatenate(
        [res.results[ci]["out"][:cfg.NL] for ci in range(ncores)], axis=0)
    return np.ascontiguousarray(out.astype(np.float32)), res


def kernel(**inputs):
    inputs = {k: np.asarray(v) for k, v in inputs.items()}
    out, _ = run_gat(
        N_NODES, NCORES, 7,
        inputs["in_feat"].astype(F32), inputs["edge_weights"].astype(F32),
        inputs["W0"], inputs["al0"], inputs["ar0"], inputs["b0"],
        inputs["W1"], inputs["al1"], inputs["ar1"], inputs["b1"],
        inputs["src"].astype(np.int64), inputs["dst"].astype(np.int64))
    return out


# revision 25
# speedup vs baseline: 1.0245x; 1.0245x over previous
"""Trainium2 Bass kernel for a 2-layer GAT with edge-weighted aggregation.

Sharding: nodes partitioned across 8 cores by dst range; edges assigned to the
core owning their dst node (host-side sort); weight matrices replicated.
Per-edge gathers use GPSIMD dma_gather; per-dst segment reductions are
selection-matrix matmuls on the TensorEngine accumulating in PSUM; node tables
are exchanged between layers via AllGather collectives.

v2: one packed per-layer table row [feat bf16 x128 | el f32 x4 | er f32 x4 |
pad] = 512B so the GAT pass needs one full-rate src gather (feat+el) plus one
dst gather of the el/er half-row; selection matrices built with single
tensor_scalar ops (per-partition scalar operands keep the DVE fast mode);
exp(leaky_relu(x)) computed as max(exp(x), exp(0.2x)) on the Scalar engine.
"""

import sys

sys.path.insert(0, "/opt/trn_rl_repo")

import numpy as np
import ml_dtypes

import concourse.bacc as bacc
import concourse.bass as bass
import concourse.mybir as mybir
import concourse.tile as tile
from concourse.bass_utils import run_bass_kernel_spmd

BF16 = ml_dtypes.bfloat16
F32 = np.float32

# full-problem constants
N_NODES = 50000
N_EDGES = 800000
D = 128
H = 4
DH = 32
NEG_SLOPE = 0.2
NCORES = 8
PROW = 256          # packed table row: 256 bf16 = 512B


class Cfg:
    def __init__(self, n_nodes, ncores, T, CB):
        self.N = n_nodes
        self.NC = ncores
        self.NL = n_nodes // ncores          # real nodes per core
        self.NBLK = -(-self.NL // 128)       # node blocks per core
        self.NLP = self.NBLK * 128           # padded nodes per core
        self.NP = ncores * self.NLP          # padded global rows
        self.GROW = (ncores // 2) * self.NLP  # padded rows per src group
        self.T = T                           # edge tiles per (group, block)
        self.CB = CB                         # blocks per gather chunk
        assert self.NBLK % CB == 0
        self.NCHUNK = self.NBLK // CB
        self.S = 2 * self.NBLK * T * 128     # edge slots per core


def _wrap_idx(idx_flat):
    """Logical idx list [S] -> [128, S/16] int16 (16-way wrap, x8 replicas)."""
    a = np.ascontiguousarray(idx_flat.reshape(-1, 16).T.astype(np.int16))
    return np.ascontiguousarray(np.tile(a, (8, 1)))


def _prep_host(cfg, in_feat, edge_weights, src, dst, b0):
    """Build all per-core host arrays."""
    NL, NLP, NBLK, T, S = cfg.NL, cfg.NLP, cfg.NBLK, cfg.T, cfg.S
    half = cfg.N // 2
    c = dst // NL
    dl = dst - c * NL
    b = dl // 128
    g = (src >= half).astype(np.int64)
    psrc = (src // NL) * NLP + (src % NL)
    gidx_val = (psrc - g * cfg.GROW).astype(np.int64)
    assert gidx_val.min() >= 0 and gidx_val.max() < cfg.GROW < 32768

    cell = ((c * NBLK + b) * 2 + g).astype(np.int64)
    # sort by (cell, src row): ascending gather addresses within each cell
    # give the SDMA engines much better HBM locality than dst order
    order = np.lexsort((gidx_val, cell))
    cell_s = cell[order]
    counts = np.bincount(cell_s, minlength=cfg.NC * NBLK * 2)
    assert counts.max() <= T * 128, (counts.max(), T * 128)
    within = np.arange(len(cell_s), dtype=np.int64)
    seg_starts = np.r_[0, np.flatnonzero(np.diff(cell_s)) + 1]
    seg_lens = np.diff(np.r_[seg_starts, len(cell_s)])
    within -= np.repeat(seg_starts, seg_lens)

    co = c[order]
    go = g[order]
    bo = b[order]
    slot = (go * NBLK + bo) * T * 128 + within  # local slot within core

    gidx_a = np.zeros((cfg.NC, S), dtype=np.int64)
    dgidx_a = np.zeros((cfg.NC, S), dtype=np.int64)
    dstin_a = np.full((cfg.NC, S), -1.0, dtype=F32)
    wts_a = np.zeros((cfg.NC, S), dtype=F32)
    gidx_a[co, slot] = gidx_val[order]
    dgidx_a[co, slot] = dl[order]
    dstin_a[co, slot] = (dl[order] - bo * 128).astype(F32)
    wts_a[co, slot] = edge_weights[order]

    per_core = []
    for ci in range(cfg.NC):
        sl = in_feat[ci * NL:(ci + 1) * NL]
        infeat = np.zeros((NLP, D), dtype=F32)
        infeat[:NL] = sl
        infeatb0 = np.zeros((NLP, D), dtype=F32)
        infeatb0[:NL] = sl + b0[None, :]
        per_core.append(dict(
            gidx=_wrap_idx(gidx_a[ci]),
            dgidx=_wrap_idx(dgidx_a[ci]),
            dstin=np.ascontiguousarray(dstin_a[ci].reshape(-1, 128).T),
            wts=np.ascontiguousarray(wts_a[ci].reshape(-1, 128).T),
            infeat=infeat,
            infeatb0=infeatb0,
        ))

    h0tab = np.zeros((cfg.NP, D), dtype=np.float16)
    for ci in range(cfg.NC):
        h0tab[ci * NLP:ci * NLP + NL] = in_feat[ci * NL:(ci + 1) * NL].astype(
            np.float16)
    return per_core, h0tab


def _alar(al, ar):
    """[H,DH] x2 -> [128, 8] f32 selector for el/er column extraction."""
    m = np.zeros((D, 2 * H), dtype=F32)
    for h in range(H):
        m[h * DH:(h + 1) * DH, h] = al[h]
        m[h * DH:(h + 1) * DH, H + h] = ar[h]
    return m


def _build(cfg, for_sim=False, stages=5, no_cc=False):
    """Build the SPMD Bass program (identical on all cores)."""
    dt = mybir.dt
    nc = bacc.Bacc("TRN2", debug=for_sim, target_bir_lowering=not for_sim,
                   num_devices=cfg.NC, num_swdge_queues=4)
    NBLK, T, CB, S, NP, NLP = cfg.NBLK, cfg.T, cfg.CB, cfg.S, cfg.NP, cfg.NLP
    GROW = cfg.GROW
    groups = [list(range(cfg.NC))]
    is_eq = mybir.AluOpType.is_equal
    mult = mybir.AluOpType.mult
    add = mybir.AluOpType.add
    amax = mybir.AluOpType.max

    # external inputs
    h0tab = nc.dram_tensor("h0tab", [NP, D], dt.float16, kind="ExternalInput")
    infeat = nc.dram_tensor("infeat", [NLP, D], dt.float32,
                            kind="ExternalInput")
    infeatb0 = nc.dram_tensor("infeatb0", [NLP, D], dt.float32,
                              kind="ExternalInput")
    w0 = nc.dram_tensor("w0", [D, D], dt.float32, kind="ExternalInput")
    w1 = nc.dram_tensor("w1", [D, D], dt.float32, kind="ExternalInput")
    alar0 = nc.dram_tensor("alar0", [D, 2 * H], dt.float32,
                           kind="ExternalInput")
    alar1 = nc.dram_tensor("alar1", [D, 2 * H], dt.float32,
                           kind="ExternalInput")
    b1t = nc.dram_tensor("b1t", [128, D], dt.float32, kind="ExternalInput")
    iota_in = nc.dram_tensor("iota", [128, 128], dt.float16,
                             kind="ExternalInput")
    ident_in = nc.dram_tensor("ident", [128, 128], dt.float32,
                              kind="ExternalInput")
    gidx = nc.dram_tensor("gidx", [128, S // 16], dt.int16,
                          kind="ExternalInput")
    dgidx = nc.dram_tensor("dgidx", [128, S // 16], dt.int16,
                           kind="ExternalInput")
    dstin_t = nc.dram_tensor("dstin", [128, S // 128], dt.float32,
                             kind="ExternalInput")
    wts_t = nc.dram_tensor("wts", [128, S // 128], dt.float32,
                           kind="ExternalInput")
    out_t = nc.dram_tensor("out", [NLP, D], dt.float32, kind="ExternalOutput")

    with tile.TileContext(nc) as tc:
        with (
            tc.tile_pool(name="dram", bufs=1, space="DRAM") as dram,
            tc.tile_pool(name="const", bufs=1) as pc,
            tc.tile_pool(name="acc", bufs=1) as pacc,
            tc.tile_pool(name="idx", bufs=3) as pidx,
            tc.tile_pool(name="gath", bufs=2) as pg,
            tc.tile_pool(name="work", bufs=3) as pw,
            tc.tile_pool(name="sel", bufs=6) as psel,
            tc.tile_pool(name="small", bufs=4) as psm,
            tc.tile_pool(name="ps", bufs=4, space="PSUM") as pps,
            tc.tile_pool(name="pss", bufs=2, space="PSUM") as ppss,
        ):
            # internal DRAM: packed per-layer tables + h2 table
            ptab_slice = [dram.tile([NLP, PROW], dt.bfloat16,
                                    name=f"ptabsl{l}") for l in range(2)]
            ptab_full = [dram.tile([NP, PROW], dt.bfloat16,
                                   addr_space="Shared", name=f"ptabfu{l}")
                         for l in range(2)]
            h2slice = dram.tile([NLP, D], dt.float16, name="h2slice")
            h2full = dram.tile([NP, D], dt.float16, addr_space="Shared",
                               name="h2full")

            # resident SBUF
            iota_sb = pc.tile([128, 128], dt.float16, name="iota_sb")
            nc.sync.dma_start(out=iota_sb[:], in_=iota_in[:])
            ident_sb = pc.tile([128, 128], dt.float32, name="ident_sb")
            nc.sync.dma_start(out=ident_sb[:], in_=ident_in[:])
            w_sb = [pc.tile([D, D], dt.float32, name=f"w_sb{l}")
                    for l in range(2)]
            nc.sync.dma_start(out=w_sb[0][:], in_=w0[:])
            nc.sync.dma_start(out=w_sb[1][:], in_=w1[:])
            alar_sb = [pc.tile([D, 2 * H], dt.float32, name=f"alar_sb{l}")
                       for l in range(2)]
            nc.sync.dma_start(out=alar_sb[0][:], in_=alar0[:])
            nc.sync.dma_start(out=alar_sb[1][:], in_=alar1[:])
            b1_sb = pc.tile([128, D], dt.float32, name="b1_sb")
            nc.sync.dma_start(out=b1_sb[:], in_=b1t[:])
            dstin_sb = pc.tile([128, S // 128], dt.float32, name="dstin_sb")
            nc.sync.dma_start(out=dstin_sb[:], in_=dstin_t[:])
            wts_sb = pc.tile([128, S // 128], dt.float32, name="wts_sb")
            nc.sync.dma_start(out=wts_sb[:], in_=wts_t[:])
            elrnm_sb = pc.tile([128, NBLK * 8], dt.float32, name="elrnm_sb")

            acc_a = pacc.tile([128, NBLK * 132], dt.float32, name="acc_a")
            acc_b = pacc.tile([128, NBLK * 132], dt.float32, name="acc_b")

            qn = [0]

            def next_q():
                return 0

            def maybe_cc(ins_ap, outs_ap):
                if no_cc:
                    return
                nc.gpsimd.collective_compute(
                    "AllGather", mybir.AluOpType.bypass,
                    replica_groups=groups, ins=[ins_ap], outs=[outs_ap])

            def agg_pass(src_tab, dest_acc, init_from_infeat,
                         after_block=None):
                """dest_acc[:, b*132:+128] (+)= sum_e w_e * tab[src_e]."""
                for bc in range(cfg.NCHUNK):
                    for g in range(2):
                        slot0 = (g * NBLK + bc * CB) * T * 128
                        nsl = CB * T * 128
                        idxt = pidx.tile([128, nsl // 16], dt.int16,
                                         name="idxa", tag="gi")
                        nc.sync.dma_start(
                            out=idxt[:],
                            in_=gidx[:, slot0 // 16:(slot0 + nsl) // 16])
                        hg = pg.tile([128, CB * T, 128], dt.float16,
                                     name="hg", tag="fgb")
                        nc.gpsimd.dma_gather(
                            hg[:], src_tab[g * GROW:(g + 1) * GROW, :],
                            idxt[:], nsl, nsl, 128, queue_num=next_q(),
                            single_packet=False)
                        for bi in range(CB):
                            b = bc * CB + bi
                            gt0 = (g * NBLK + b) * T
                            ps = pps.tile([128, 128], dt.float32, name="psa",
                                          tag="scat")
                            for t in range(T):
                                selw = psel.tile([128, 128], dt.float16,
                                                 name="selw", tag="sel")
                                nc.vector.tensor_scalar(
                                    out=selw[:], in0=iota_sb[:],
                                    scalar1=dstin_sb[:, gt0 + t:gt0 + t + 1],
                                    scalar2=wts_sb[:, gt0 + t:gt0 + t + 1],
                                    op0=is_eq, op1=mult)
                                nc.tensor.matmul(
                                    ps[:], selw[:],
                                    hg[:, bi * T + t, :],
                                    start=(t == 0), stop=(t == T - 1))
                            dsl = dest_acc[:, b * 132:b * 132 + 128]
                            if init_from_infeat and g == 0:
                                it = psm.tile([128, 128], dt.float32,
                                              name="ift", tag="ift")
                                nc.sync.dma_start(
                                    out=it[:],
                                    in_=infeat[b * 128:(b + 1) * 128, :])
                                nc.vector.tensor_tensor(
                                    out=dsl, in0=ps[:], in1=it[:], op=add)
                            else:
                                nc.vector.tensor_tensor(
                                    out=dsl, in0=ps[:], in1=dsl, op=add)
                            if g == 1 and after_block is not None:
                                after_block(b)

            def feat_block(src_acc, layer, b):
                    hsl = src_acc[:, b * 132:b * 132 + 128]
                    tp = pps.tile([128, 128], dt.float32, name="tp",
                                  tag="scat")
                    nc.tensor.transpose(out=tp[:], in_=hsl,
                                        identity=ident_sb[:])
                    hT = psm.tile([128, 128], dt.float32, name="hT", tag="hT")
                    nc.vector.tensor_copy(out=hT[:], in_=tp[:])
                    fp = pps.tile([128, 128], dt.float32, name="fp",
                                  tag="scat")
                    nc.tensor.matmul(fp[:], w_sb[layer][:], hT[:],
                                     start=True, stop=True)
                    fT = psm.tile([128, 128], dt.float32, name="fT", tag="fT")
                    nc.vector.tensor_copy(out=fT[:], in_=fp[:])
                    ep = ppss.tile([2 * H, 128], dt.float32, name="ep",
                                   tag="ep")
                    nc.tensor.matmul(ep[:], alar_sb[layer][:], fT[:],
                                     start=True, stop=True)
                    eT = psm.tile([2 * H, 128], dt.float32, name="eT",
                                  tag="eT")
                    nc.vector.tensor_copy(out=eT[:], in_=ep[:])
                    enm = ppss.tile([128, 2 * H], dt.float32, name="enm",
                                    tag="ep")
                    nc.tensor.matmul(enm[:], eT[:],
                                     ident_sb[0:2 * H, 0:2 * H],
                                     start=True, stop=True)
                    nc.vector.tensor_copy(out=elrnm_sb[:, b * 8:(b + 1) * 8],
                                          in_=enm[:])
                    ftp = pps.tile([128, 128], dt.float32, name="ftp",
                                   tag="scat")
                    nc.tensor.transpose(out=ftp[:], in_=fT[:],
                                        identity=ident_sb[:])
                    fnm = psm.tile([128, 128], dt.bfloat16, name="fnm",
                                   tag="fnm")
                    nc.vector.tensor_copy(out=fnm[:], in_=ftp[:])
                    nc.sync.dma_start(
                        out=ptab_slice[layer][b * 128:(b + 1) * 128, 0:D],
                        in_=fnm[:])

            def feat_tail(layer):
                # el/er columns: f32 values bit-cast into the bf16 table
                nc.sync.dma_start(
                    out=ptab_slice[layer][:, D:D + 16].rearrange(
                        "(b p) c -> p b c", p=128),
                    in_=elrnm_sb[:].bitcast(dt.bfloat16).rearrange(
                        "p (b c) -> p b c", c=16))
                maybe_cc(ptab_slice[layer].opt(), ptab_full[layer].opt())

            def gat_pass(layer, dest_acc, after_block=None):
                """GAT message pass accumulating [usum | s] into dest_acc."""
                for bc in range(cfg.NCHUNK):
                    for g in range(2):
                        slot0 = (g * NBLK + bc * CB) * T * 128
                        nsl = CB * T * 128
                        idxt = pidx.tile([128, nsl // 16], dt.int16,
                                         name="idxg", tag="gi")
                        nc.sync.dma_start(
                            out=idxt[:],
                            in_=gidx[:, slot0 // 16:(slot0 + nsl) // 16])
                        didxt = pidx.tile([128, nsl // 16], dt.int16,
                                          name="idxd", tag="di")
                        nc.sync.dma_start(
                            out=didxt[:],
                            in_=dgidx[:, slot0 // 16:(slot0 + nsl) // 16])
                        fg = pg.tile([128, CB * T, PROW], dt.bfloat16,
                                     name="fg", tag="fgb")
                        nc.gpsimd.dma_gather(
                            fg[:],
                            ptab_full[layer][g * GROW:(g + 1) * GROW, :],
                            idxt[:], nsl, nsl, PROW, queue_num=next_q(),
                            single_packet=False)
                        eg2 = pg.tile([128, CB * T, 128], dt.bfloat16,
                                      name="eg2", tag="eg2")
                        nc.gpsimd.dma_gather(
                            eg2[:], ptab_slice[layer][:, D:PROW],
                            didxt[:], nsl, nsl, 128, elem_step=PROW,
                            queue_num=next_q(), single_packet=False)
                        # e = el[src] + er[dst]; ee = max(exp(e), exp(.2 e))
                        ev = pw.tile([128, CB * T * H], dt.float32,
                                     name="ev", tag="ev")
                        nc.vector.tensor_tensor(
                            out=ev[:].rearrange("p (c h) -> p c h", h=H),
                            in0=fg[:, :, D:D + 8].bitcast(dt.float32),
                            in1=eg2[:, :, 8:16].bitcast(dt.float32),
                            op=add)
                        e1 = pw.tile([128, CB * T * H], dt.float32,
                                     name="e1", tag="e1")
                        nc.scalar.activation(
                            out=e1[:], in_=ev[:],
                            func=mybir.ActivationFunctionType.Exp)
                        e2 = pw.tile([128, CB * T * H], dt.float32,
                                     name="e2", tag="e2")
                        nc.scalar.activation(
                            out=e2[:], in_=ev[:],
                            func=mybir.ActivationFunctionType.Exp,
                            scale=NEG_SLOPE)
                        eeb = pw.tile([128, CB * T * H], dt.bfloat16,
                                      name="eeb", tag="eeb")
                        nc.vector.tensor_tensor(out=eeb[:], in0=e1[:],
                                                in1=e2[:], op=amax)
                        for bi in range(CB):
                            b = bc * CB + bi
                            gt0 = (g * NBLK + b) * T
                            rsc = pw.tile([128, T * 128], dt.bfloat16,
                                          name="rsc", tag="rsc")
                            for t in range(T):
                                c = bi * T + t
                                nc.vector.tensor_tensor(
                                    out=rsc[:, t * 128:(t + 1) * 128
                                            ].rearrange(
                                        "p (h d) -> p h d", h=H),
                                    in0=fg[:, c, 0:D].rearrange(
                                        "p (h d) -> p h d", h=H),
                                    in1=eeb[:, c * H:(c + 1) * H, None
                                            ].to_broadcast([128, H, DH]),
                                    op=mult)
                            ps = pps.tile([128, 128], dt.float32, name="psg",
                                          tag="scat")
                            ps_s = ppss.tile([128, H], dt.float32,
                                             name="pss", tag="ep")
                            for t in range(T):
                                sel = psel.tile([128, 128], dt.bfloat16,
                                                name="selg", tag="sel")
                                nc.vector.tensor_scalar(
                                    out=sel[:], in0=iota_sb[:],
                                    scalar1=dstin_sb[:, gt0 + t:gt0 + t + 1],
                                    scalar2=None, op0=is_eq)
                                nc.tensor.matmul(
                                    ps[:], sel[:],
                                    rsc[:, t * 128:(t + 1) * 128],
                                    start=(t == 0), stop=(t == T - 1))
                                nc.tensor.matmul(
                                    ps_s[:], sel[:],
                                    eeb[:, (bi * T + t) * H:
                                        (bi * T + t + 1) * H],
                                    start=(t == 0), stop=(t == T - 1))
                            dsl = dest_acc[:, b * 132:b * 132 + 128]
                            dss = dest_acc[:, b * 132 + 128:b * 132 + 132]
                            if g == 0:
                                nc.vector.tensor_copy(out=dsl, in_=ps[:])
                                nc.vector.tensor_copy(out=dss, in_=ps_s[:])
                            else:
                                nc.vector.tensor_tensor(
                                    out=dsl, in0=ps[:], in1=dsl, op=add)
                                nc.vector.tensor_tensor(
                                    out=dss, in0=ps_s[:], in1=dss, op=add)
                                if after_block is not None:
                                    after_block(b)

            def fin1(b):
                ssb = psm.tile([128, H], dt.float32, name="ssb", tag="ssb")
                nc.vector.tensor_scalar_add(
                    out=ssb[:], in0=acc_a[:, b * 132 + 128:b * 132 + 132],
                    scalar1=1e-30)
                sr = psm.tile([128, H], dt.float32, name="sr", tag="sr")
                nc.vector.reciprocal(out=sr[:], in_=ssb[:])
                rst = psm.tile([128, 128], dt.float32, name="rst", tag="rst")
                nc.vector.tensor_tensor(
                    out=rst[:].rearrange("p (h d) -> p h d", h=H),
                    in0=acc_a[:, b * 132:b * 132 + 128].rearrange(
                        "p (h d) -> p h d", h=H),
                    in1=sr[:, :, None].to_broadcast([128, H, DH]),
                    op=mult)
                it = psm.tile([128, 128], dt.float32, name="ifb", tag="ift")
                nc.sync.dma_start(out=it[:],
                                  in_=infeatb0[b * 128:(b + 1) * 128, :])
                hb = psm.tile([128, 128], dt.float32, name="hb", tag="hb")
                nc.vector.tensor_tensor(out=hb[:], in0=rst[:], in1=it[:],
                                        op=add)
                nc.scalar.activation(
                    out=acc_b[:, b * 132:b * 132 + 128], in_=hb[:],
                    func=mybir.ActivationFunctionType.Relu)
                h2b = psm.tile([128, 128], dt.float16, name="h2b", tag="h2b")
                nc.vector.tensor_copy(out=h2b[:],
                                      in_=acc_b[:, b * 132:b * 132 + 128])
                nc.sync.dma_start(out=h2slice[b * 128:(b + 1) * 128, :],
                                  in_=h2b[:])

            def fin2(b):
                ssb = psm.tile([128, H], dt.float32, name="ssb2", tag="ssb")
                nc.vector.tensor_scalar_add(
                    out=ssb[:], in0=acc_a[:, b * 132 + 128:b * 132 + 132],
                    scalar1=1e-30)
                sr = psm.tile([128, H], dt.float32, name="sr2", tag="sr")
                nc.vector.reciprocal(out=sr[:], in_=ssb[:])
                rst = psm.tile([128, 128], dt.float32, name="rst2", tag="rst")
                nc.vector.tensor_tensor(
                    out=rst[:].rearrange("p (h d) -> p h d", h=H),
                    in0=acc_a[:, b * 132:b * 132 + 128].rearrange(
                        "p (h d) -> p h d", h=H),
                    in1=sr[:, :, None].to_broadcast([128, H, DH]),
                    op=mult)
                ob = psm.tile([128, 128], dt.float32, name="ob", tag="hb")
                nc.vector.tensor_tensor(out=ob[:], in0=rst[:], in1=b1_sb[:],
                                        op=add)
                nc.sync.dma_start(out=out_t[b * 128:(b + 1) * 128, :],
                                  in_=ob[:])

            # ---------------- layer 1 ----------------
            agg_pass(h0tab, acc_a, init_from_infeat=True,
                     after_block=(lambda b: feat_block(acc_a, 0, b))
                     if stages >= 2 else None)
            if stages >= 2:
                feat_tail(0)
            if stages >= 3:
                gat_pass(0, acc_a, after_block=fin1)
                maybe_cc(h2slice.opt(), h2full.opt())

            # ---------------- layer 2 ----------------
            if stages >= 4:
                agg_pass(h2full, acc_b, init_from_infeat=False,
                         after_block=lambda b: feat_block(acc_b, 1, b))
                feat_tail(1)
            if stages >= 5:
                gat_pass(1, acc_a, after_block=fin2)
            if stages < 5:
                src_acc = acc_a if stages < 4 else acc_b
                for b in range(NBLK):
                    nc.sync.dma_start(
                        out=out_t[b * 128:(b + 1) * 128, :],
                        in_=src_acc[:, b * 132:b * 132 + 128])

    nc.compile()
    return nc


def _compute_T(cfg_n, ncores, src, dst):
    NL = cfg_n // ncores
    NBLK = -(-NL // 128)
    c = dst // NL
    dl = dst - c * NL
    b = dl // 128
    g = (src >= cfg_n // 2).astype(np.int64)
    cell = (c * NBLK + b) * 2 + g
    cnt = np.bincount(cell, minlength=ncores * NBLK * 2)
    return int(-(-cnt.max() // 128))


def run_gat(n_nodes, ncores, CB, in_feat, edge_weights, W0, al0, ar0, b0,
            W1, al1, ar1, b1, src, dst, trace=False):
    """Shared entry: build, run on hardware, return full output."""
    T = _compute_T(n_nodes, ncores, src, dst)
    cfg = Cfg(n_nodes, ncores, T, CB)
    per_core, h0tab = _prep_host(cfg, in_feat, edge_weights, src, dst, b0)
    nc = _build(cfg)

    iota = np.tile(np.arange(128, dtype=F32)[None, :], (128, 1)).astype(
        np.float16)
    ident = np.eye(128, dtype=F32)
    shared = dict(
        h0tab=h0tab, w0=W0.astype(F32), w1=W1.astype(F32),
        alar0=_alar(al0, ar0), alar1=_alar(al1, ar1),
        b1t=np.tile(b1[None, :], (128, 1)).astype(F32),
        iota=np.ascontiguousarray(iota), ident=ident,
    )
    in_maps = []
    for ci in range(ncores):
        m = dict(shared)
        m.update(per_core[ci])
        in_maps.append(m)
    res = run_bass_kernel_spmd(nc, in_maps, core_ids=list(range(ncores)),
                               trace=trace)
    out = np.concatenate(
        [res.results[ci]["out"][:cfg.NL] for ci in range(ncores)], axis=0)
    return np.ascontiguousarray(out.astype(np.float32)), res


def kernel(**inputs):
    inputs = {k: np.asarray(v) for k, v in inputs.items()}
    out, _ = run_gat(
        N_NODES, NCORES, 7,
        inputs["in_feat"].astype(F32), inputs["edge_weights"].astype(F32),
        inputs["W0"], inputs["al0"], inputs["ar0"], inputs["b0"],
        inputs["W1"], inputs["al1"], inputs["ar1"], inputs["b1"],
        inputs["src"].astype(np.int64), inputs["dst"].astype(np.int64))
    return out


# revision 26
# speedup vs baseline: 1.0988x; 1.0726x over previous
"""Trainium2 Bass kernel for a 2-layer GAT with edge-weighted aggregation.

Sharding: nodes partitioned across 8 cores by dst range; edges assigned to the
core owning their dst node (host-side sort); weight matrices replicated.
Per-edge gathers use GPSIMD dma_gather; per-dst segment reductions are
selection-matrix matmuls on the TensorEngine accumulating in PSUM; node tables
are exchanged between layers via AllGather collectives.

v2: one packed per-layer table row [feat bf16 x128 | el f32 x4 | er f32 x4 |
pad] = 512B so the GAT pass needs one full-rate src gather (feat+el) plus one
dst gather of the el/er half-row; selection matrices built with single
tensor_scalar ops (per-partition scalar operands keep the DVE fast mode);
exp(leaky_relu(x)) computed as max(exp(x), exp(0.2x)) on the Scalar engine.
"""

import sys

sys.path.insert(0, "/opt/trn_rl_repo")

import numpy as np
import ml_dtypes

import concourse.bacc as bacc
import concourse.bass as bass
import concourse.mybir as mybir
import concourse.tile as tile
from concourse.bass_utils import run_bass_kernel_spmd

BF16 = ml_dtypes.bfloat16
F32 = np.float32

# full-problem constants
N_NODES = 50000
N_EDGES = 800000
D = 128
H = 4
DH = 32
NEG_SLOPE = 0.2
NCORES = 8
PROW = 256          # packed table row: 256 bf16 = 512B


class Cfg:
    def __init__(self, n_nodes, ncores, T, CB):
        self.N = n_nodes
        self.NC = ncores
        self.NL = n_nodes // ncores          # real nodes per core
        self.NBLK = -(-self.NL // 128)       # node blocks per core
        self.NLP = self.NBLK * 128           # padded nodes per core
        self.NP = ncores * self.NLP          # padded global rows
        self.GROW = (ncores // 2) * self.NLP  # padded rows per src group
        self.T = T                           # edge tiles per (group, block)
        self.CB = CB                         # blocks per gather chunk
        assert self.NBLK % CB == 0
        self.NCHUNK = self.NBLK // CB
        self.S = 2 * self.NBLK * T * 128     # edge slots per core


def _wrap_idx(idx_flat):
    """Logical idx list [S] -> [128, S/16] int16 (16-way wrap, x8 replicas)."""
    a = np.ascontiguousarray(idx_flat.reshape(-1, 16).T.astype(np.int16))
    return np.ascontiguousarray(np.tile(a, (8, 1)))


def _prep_host(cfg, in_feat, edge_weights, src, dst, b0):
    """Build all per-core host arrays."""
    NL, NLP, NBLK, T, S = cfg.NL, cfg.NLP, cfg.NBLK, cfg.T, cfg.S
    half = cfg.N // 2
    c = dst // NL
    dl = dst - c * NL
    b = dl // 128
    g = (src >= half).astype(np.int64)
    psrc = (src // NL) * NLP + (src % NL)
    gidx_val = (psrc - g * cfg.GROW).astype(np.int64)
    assert gidx_val.min() >= 0 and gidx_val.max() < cfg.GROW < 32768

    cell = ((c * NBLK + b) * 2 + g).astype(np.int64)
    # sort by (cell, src row): ascending gather addresses within each cell
    # give the SDMA engines much better HBM locality than dst order
    order = np.lexsort((gidx_val, cell))
    cell_s = cell[order]
    counts = np.bincount(cell_s, minlength=cfg.NC * NBLK * 2)
    assert counts.max() <= T * 128, (counts.max(), T * 128)
    within = np.arange(len(cell_s), dtype=np.int64)
    seg_starts = np.r_[0, np.flatnonzero(np.diff(cell_s)) + 1]
    seg_lens = np.diff(np.r_[seg_starts, len(cell_s)])
    within -= np.repeat(seg_starts, seg_lens)

    co = c[order]
    go = g[order]
    bo = b[order]
    slot = (go * NBLK + bo) * T * 128 + within  # local slot within core

    gidx_a = np.zeros((cfg.NC, S), dtype=np.int64)
    dgidx_a = np.zeros((cfg.NC, S), dtype=np.int64)
    dstin_a = np.full((cfg.NC, S), -1.0, dtype=F32)
    wts_a = np.zeros((cfg.NC, S), dtype=F32)
    gidx_a[co, slot] = gidx_val[order]
    dgidx_a[co, slot] = dl[order]
    dstin_a[co, slot] = (dl[order] - bo * 128).astype(F32)
    wts_a[co, slot] = edge_weights[order]

    per_core = []
    for ci in range(cfg.NC):
        sl = in_feat[ci * NL:(ci + 1) * NL]
        infeat = np.zeros((NLP, D), dtype=F32)
        infeat[:NL] = sl
        infeatb0 = np.zeros((NLP, D), dtype=F32)
        infeatb0[:NL] = sl + b0[None, :]
        per_core.append(dict(
            gidx=_wrap_idx(gidx_a[ci]),
            dgidx=_wrap_idx(dgidx_a[ci]),
            dstin=np.ascontiguousarray(dstin_a[ci].reshape(-1, 128).T),
            wts=np.ascontiguousarray(wts_a[ci].reshape(-1, 128).T),
            infeat=infeat,
            infeatb0=infeatb0,
        ))

    h0tab = np.zeros((cfg.NP, D), dtype=np.float16)
    for ci in range(cfg.NC):
        h0tab[ci * NLP:ci * NLP + NL] = in_feat[ci * NL:(ci + 1) * NL].astype(
            np.float16)
    return per_core, h0tab


def _alar(al, ar):
    """[H,DH] x2 -> [128, 8] f32 selector for el/er column extraction."""
    m = np.zeros((D, 2 * H), dtype=F32)
    for h in range(H):
        m[h * DH:(h + 1) * DH, h] = al[h]
        m[h * DH:(h + 1) * DH, H + h] = ar[h]
    return m


def _build(cfg, for_sim=False, stages=5, no_cc=False):
    """Build the SPMD Bass program (identical on all cores)."""
    dt = mybir.dt
    nc = bacc.Bacc("TRN2", debug=for_sim, target_bir_lowering=not for_sim,
                   num_devices=cfg.NC, num_swdge_queues=4)
    NBLK, T, CB, S, NP, NLP = cfg.NBLK, cfg.T, cfg.CB, cfg.S, cfg.NP, cfg.NLP
    GROW = cfg.GROW
    groups = [list(range(cfg.NC))]
    is_eq = mybir.AluOpType.is_equal
    mult = mybir.AluOpType.mult
    add = mybir.AluOpType.add
    amax = mybir.AluOpType.max

    # external inputs
    h0tab = nc.dram_tensor("h0tab", [NP, D], dt.float16, kind="ExternalInput")
    infeat = nc.dram_tensor("infeat", [NLP, D], dt.float32,
                            kind="ExternalInput")
    infeatb0 = nc.dram_tensor("infeatb0", [NLP, D], dt.float32,
                              kind="ExternalInput")
    w0 = nc.dram_tensor("w0", [D, D], dt.float32, kind="ExternalInput")
    w1 = nc.dram_tensor("w1", [D, D], dt.float32, kind="ExternalInput")
    alar0 = nc.dram_tensor("alar0", [D, 2 * H], dt.float32,
                           kind="ExternalInput")
    alar1 = nc.dram_tensor("alar1", [D, 2 * H], dt.float32,
                           kind="ExternalInput")
    b1t = nc.dram_tensor("b1t", [128, D], dt.float32, kind="ExternalInput")
    iota_in = nc.dram_tensor("iota", [128, 128], dt.float16,
                             kind="ExternalInput")
    ident_in = nc.dram_tensor("ident", [128, 128], dt.float32,
                              kind="ExternalInput")
    gidx = nc.dram_tensor("gidx", [128, S // 16], dt.int16,
                          kind="ExternalInput")
    dgidx = nc.dram_tensor("dgidx", [128, S // 16], dt.int16,
                           kind="ExternalInput")
    dstin_t = nc.dram_tensor("dstin", [128, S // 128], dt.float32,
                             kind="ExternalInput")
    wts_t = nc.dram_tensor("wts", [128, S // 128], dt.float32,
                           kind="ExternalInput")
    out_t = nc.dram_tensor("out", [NLP, D], dt.float32, kind="ExternalOutput")

    with tile.TileContext(nc) as tc:
        with (
            tc.tile_pool(name="dram", bufs=1, space="DRAM") as dram,
            tc.tile_pool(name="const", bufs=1) as pc,
            tc.tile_pool(name="acc", bufs=1) as pacc,
            tc.tile_pool(name="idx", bufs=3) as pidx,
            tc.tile_pool(name="gath", bufs=2) as pg,
            tc.tile_pool(name="work", bufs=3) as pw,
            tc.tile_pool(name="sel", bufs=6) as psel,
            tc.tile_pool(name="small", bufs=4) as psm,
            tc.tile_pool(name="ps", bufs=4, space="PSUM") as pps,
            tc.tile_pool(name="pss", bufs=2, space="PSUM") as ppss,
        ):
            # internal DRAM: packed per-layer tables + h2 table
            ptab_slice = [dram.tile([NLP, PROW], dt.bfloat16,
                                    name=f"ptabsl{l}") for l in range(2)]
            ptab_full = [dram.tile([NP, PROW], dt.bfloat16,
                                   addr_space="Shared", name=f"ptabfu{l}")
                         for l in range(2)]
            h2slice = dram.tile([NLP, D], dt.float16, name="h2slice")
            h2full = dram.tile([NP, D], dt.float16, addr_space="Shared",
                               name="h2full")

            # resident SBUF
            iota_sb = pc.tile([128, 128], dt.float16, name="iota_sb")
            nc.sync.dma_start(out=iota_sb[:], in_=iota_in[:])
            ident_sb = pc.tile([128, 128], dt.float32, name="ident_sb")
            nc.sync.dma_start(out=ident_sb[:], in_=ident_in[:])
            w_sb = [pc.tile([D, D], dt.float32, name=f"w_sb{l}")
                    for l in range(2)]
            nc.sync.dma_start(out=w_sb[0][:], in_=w0[:])
            nc.sync.dma_start(out=w_sb[1][:], in_=w1[:])
            alar_sb = [pc.tile([D, 2 * H], dt.float32, name=f"alar_sb{l}")
                       for l in range(2)]
            nc.sync.dma_start(out=alar_sb[0][:], in_=alar0[:])
            nc.sync.dma_start(out=alar_sb[1][:], in_=alar1[:])
            b1_sb = pc.tile([128, D], dt.float32, name="b1_sb")
            nc.sync.dma_start(out=b1_sb[:], in_=b1t[:])
            dstin_sb = pc.tile([128, S // 128], dt.float32, name="dstin_sb")
            nc.sync.dma_start(out=dstin_sb[:], in_=dstin_t[:])
            wts_sb = pc.tile([128, S // 128], dt.float32, name="wts_sb")
            nc.sync.dma_start(out=wts_sb[:], in_=wts_t[:])
            elrnm_sb = pc.tile([128, NBLK * 8], dt.float32, name="elrnm_sb")

            acc_a = pacc.tile([128, NBLK * 132], dt.float32, name="acc_a")
            acc_b = pacc.tile([128, NBLK * 132], dt.float32, name="acc_b")

            qn = [0]

            def next_q():
                return 0

            def maybe_cc(ins_ap, outs_ap):
                if no_cc:
                    return
                nc.gpsimd.collective_compute(
                    "AllGather", mybir.AluOpType.bypass,
                    replica_groups=groups, ins=[ins_ap], outs=[outs_ap])

            def agg_pass(src_tab, dest_acc, init_from_infeat,
                         after_block=None):
                """dest_acc[:, b*132:+128] (+)= sum_e w_e * tab[src_e]."""
                for bc in range(cfg.NCHUNK):
                    for g in range(2):
                        slot0 = (g * NBLK + bc * CB) * T * 128
                        nsl = CB * T * 128
                        idxt = pidx.tile([128, nsl // 16], dt.int16,
                                         name="idxa", tag="gi")
                        nc.sync.dma_start(
                            out=idxt[:],
                            in_=gidx[:, slot0 // 16:(slot0 + nsl) // 16])
                        hg = pg.tile([128, CB * T, 128], dt.float16,
                                     name="hg", tag="fgb")
                        nc.gpsimd.dma_gather(
                            hg[:], src_tab[g * GROW:(g + 1) * GROW, :],
                            idxt[:], nsl, nsl, 128, queue_num=next_q(),
                            single_packet=False)
                        for bi in range(CB):
                            b = bc * CB + bi
                            gt0 = (g * NBLK + b) * T
                            ps = pps.tile([128, 128], dt.float32, name="psa",
                                          tag="scat")
                            for t in range(T):
                                selw = psel.tile([128, 128], dt.float16,
                                                 name="selw", tag="sel")
                                nc.vector.tensor_scalar(
                                    out=selw[:], in0=iota_sb[:],
                                    scalar1=dstin_sb[:, gt0 + t:gt0 + t + 1],
                                    scalar2=wts_sb[:, gt0 + t:gt0 + t + 1],
                                    op0=is_eq, op1=mult)
                                nc.tensor.matmul(
                                    ps[:], selw[:],
                                    hg[:, bi * T + t, :],
                                    start=(t == 0), stop=(t == T - 1))
                            dsl = dest_acc[:, b * 132:b * 132 + 128]
                            if init_from_infeat and g == 0:
                                it = psm.tile([128, 128], dt.float32,
                                              name="ift", tag="ift")
                                nc.sync.dma_start(
                                    out=it[:],
                                    in_=infeat[b * 128:(b + 1) * 128, :])
                                nc.vector.tensor_tensor(
                                    out=dsl, in0=ps[:], in1=it[:], op=add)
                            else:
                                nc.vector.tensor_tensor(
                                    out=dsl, in0=ps[:], in1=dsl, op=add)
                            if g == 1 and after_block is not None:
                                after_block(b)

            def feat_block(src_acc, layer, b):
                    hsl = src_acc[:, b * 132:b * 132 + 128]
                    tp = pps.tile([128, 128], dt.float32, name="tp",
                                  tag="scat")
                    nc.tensor.transpose(out=tp[:], in_=hsl,
                                        identity=ident_sb[:])
                    hT = psm.tile([128, 128], dt.float32, name="hT", tag="hT")
                    nc.vector.tensor_copy(out=hT[:], in_=tp[:])
                    fp = pps.tile([128, 128], dt.float32, name="fp",
                                  tag="scat")
                    nc.tensor.matmul(fp[:], w_sb[layer][:], hT[:],
                                     start=True, stop=True)
                    fT = psm.tile([128, 128], dt.float32, name="fT", tag="fT")
                    nc.vector.tensor_copy(out=fT[:], in_=fp[:])
                    ep = ppss.tile([2 * H, 128], dt.float32, name="ep",
                                   tag="ep")
                    nc.tensor.matmul(ep[:], alar_sb[layer][:], fT[:],
                                     start=True, stop=True)
                    eT = psm.tile([2 * H, 128], dt.float32, name="eT",
                                  tag="eT")
                    nc.vector.tensor_copy(out=eT[:], in_=ep[:])
                    enm = ppss.tile([128, 2 * H], dt.float32, name="enm",
                                    tag="ep")
                    nc.tensor.matmul(enm[:], eT[:],
                                     ident_sb[0:2 * H, 0:2 * H],
                                     start=True, stop=True)
                    nc.vector.tensor_copy(out=elrnm_sb[:, b * 8:(b + 1) * 8],
                                          in_=enm[:])
                    ftp = pps.tile([128, 128], dt.float32, name="ftp",
                                   tag="scat")
                    nc.tensor.transpose(out=ftp[:], in_=fT[:],
                                        identity=ident_sb[:])
                    fnm = psm.tile([128, 128], dt.bfloat16, name="fnm",
                                   tag="fnm")
                    nc.vector.tensor_copy(out=fnm[:], in_=ftp[:])
                    nc.sync.dma_start(
                        out=ptab_slice[layer][b * 128:(b + 1) * 128, 0:D],
                        in_=fnm[:])

            def feat_tail(layer):
                # el/er columns: f32 values bit-cast into the bf16 table
                nc.sync.dma_start(
                    out=ptab_slice[layer][:, D:D + 16].rearrange(
                        "(b p) c -> p b c", p=128),
                    in_=elrnm_sb[:].bitcast(dt.bfloat16).rearrange(
                        "p (b c) -> p b c", c=16))
                maybe_cc(ptab_slice[layer].opt(), ptab_full[layer].opt())

            def gat_pass(layer, dest_acc, after_block=None):
                """GAT message pass accumulating [usum | s] into dest_acc."""
                for bc in range(cfg.NCHUNK):
                    for g in range(2):
                        slot0 = (g * NBLK + bc * CB) * T * 128
                        nsl = CB * T * 128
                        idxt = pidx.tile([128, nsl // 16], dt.int16,
                                         name="idxg", tag="gi")
                        nc.sync.dma_start(
                            out=idxt[:],
                            in_=gidx[:, slot0 // 16:(slot0 + nsl) // 16])
                        didxt = pidx.tile([128, nsl // 16], dt.int16,
                                          name="idxd", tag="di")
                        nc.sync.dma_start(
                            out=didxt[:],
                            in_=dgidx[:, slot0 // 16:(slot0 + nsl) // 16])
                        fg = pg.tile([128, CB * T, PROW], dt.bfloat16,
                                     name="fg", tag="fgb")
                        nc.gpsimd.dma_gather(
                            fg[:],
                            ptab_full[layer][g * GROW:(g + 1) * GROW, :],
                            idxt[:], nsl, nsl, PROW, queue_num=next_q(),
                            single_packet=False)
                        eg2 = pg.tile([128, CB * T, 128], dt.bfloat16,
                                      name="eg2", tag="eg2")
                        nc.gpsimd.dma_gather(
                            eg2[:], ptab_slice[layer][:, D:PROW],
                            didxt[:], nsl, nsl, 128, elem_step=PROW,
                            queue_num=next_q(), single_packet=False)
                        # e = el[src] + er[dst]; ee = max(exp(e), exp(.2 e))
                        ev = pw.tile([128, CB * T * H], dt.float32,
                                     name="ev", tag="ev")
                        nc.vector.tensor_tensor(
                            out=ev[:].rearrange("p (c h) -> p c h", h=H),
                            in0=fg[:, :, D:D + 8].bitcast(dt.float32),
                            in1=eg2[:, :, 8:16].bitcast(dt.float32),
                            op=add)
                        for bi in range(CB):
                            b = bc * CB + bi
                            gt0 = (g * NBLK + b) * T
                            # exp(e) expanded to 32 cols/head on the Scalar
                            # engine (broadcast input AP) so the max and the
                            # feat scaling run in the DVE bf16 4x fast mode
                            evs = ev[:, bi * T * H:(bi + 1) * T * H, None
                                     ].to_broadcast([128, T * H, DH])
                            e1x = pw.tile([128, T * 128], dt.bfloat16,
                                          name="e1x", tag="e1x")
                            nc.scalar.activation(
                                out=e1x[:].rearrange(
                                    "p (th d) -> p th d", d=DH),
                                in_=evs,
                                func=mybir.ActivationFunctionType.Exp)
                            e2x = pw.tile([128, T * 128], dt.bfloat16,
                                          name="e2x", tag="e2x")
                            nc.scalar.activation(
                                out=e2x[:].rearrange(
                                    "p (th d) -> p th d", d=DH),
                                in_=evs,
                                func=mybir.ActivationFunctionType.Exp,
                                scale=NEG_SLOPE)
                            # ee = max(exp(e), exp(.2e)), expanded; then
                            # rsc = feat * ee -- both 4x-mode ops
                            nc.vector.tensor_tensor(out=e1x[:], in0=e1x[:],
                                                    in1=e2x[:], op=amax)
                            rsc = e2x
                            nc.vector.tensor_tensor(
                                out=rsc[:].rearrange("p (t n) -> p t n", t=T),
                                in0=fg[:, bi * T:(bi + 1) * T, 0:D],
                                in1=e1x[:].rearrange("p (t n) -> p t n", t=T),
                                op=mult)
                            ps = pps.tile([128, 128], dt.float32, name="psg",
                                          tag="scat")
                            ps_s = ppss.tile([128, H], dt.float32,
                                             name="pss", tag="ep")
                            for t in range(T):
                                sel = psel.tile([128, 128], dt.bfloat16,
                                                name="selg", tag="sel")
                                nc.vector.tensor_scalar(
                                    out=sel[:], in0=iota_sb[:],
                                    scalar1=dstin_sb[:, gt0 + t:gt0 + t + 1],
                                    scalar2=None, op0=is_eq)
                                nc.tensor.matmul(
                                    ps[:], sel[:],
                                    rsc[:, t * 128:(t + 1) * 128],
                                    start=(t == 0), stop=(t == T - 1))
                                nc.tensor.matmul(
                                    ps_s[:], sel[:],
                                    e1x[:].rearrange(
                                        "p (th d) -> p th d", d=DH)[
                                        :, t * H:(t + 1) * H, 0:1],
                                    start=(t == 0), stop=(t == T - 1))
                            dsl = dest_acc[:, b * 132:b * 132 + 128]
                            dss = dest_acc[:, b * 132 + 128:b * 132 + 132]
                            if g == 0:
                                nc.vector.tensor_copy(out=dsl, in_=ps[:])
                                nc.vector.tensor_copy(out=dss, in_=ps_s[:])
                            else:
                                nc.vector.tensor_tensor(
                                    out=dsl, in0=ps[:], in1=dsl, op=add)
                                nc.vector.tensor_tensor(
                                    out=dss, in0=ps_s[:], in1=dss, op=add)
                                if after_block is not None:
                                    after_block(b)

            def fin1(b):
                ssb = psm.tile([128, H], dt.float32, name="ssb", tag="ssb")
                nc.vector.tensor_scalar_add(
                    out=ssb[:], in0=acc_a[:, b * 132 + 128:b * 132 + 132],
                    scalar1=1e-30)
                sr = psm.tile([128, H], dt.float32, name="sr", tag="sr")
                nc.vector.reciprocal(out=sr[:], in_=ssb[:])
                rst = psm.tile([128, 128], dt.float32, name="rst", tag="rst")
                nc.vector.tensor_tensor(
                    out=rst[:].rearrange("p (h d) -> p h d", h=H),
                    in0=acc_a[:, b * 132:b * 132 + 128].rearrange(
                        "p (h d) -> p h d", h=H),
                    in1=sr[:, :, None].to_broadcast([128, H, DH]),
                    op=mult)
                it = psm.tile([128, 128], dt.float32, name="ifb", tag="ift")
                nc.sync.dma_start(out=it[:],
                                  in_=infeatb0[b * 128:(b + 1) * 128, :])
                hb = psm.tile([128, 128], dt.float32, name="hb", tag="hb")
                nc.vector.tensor_tensor(out=hb[:], in0=rst[:], in1=it[:],
                                        op=add)
                nc.scalar.activation(
                    out=acc_b[:, b * 132:b * 132 + 128], in_=hb[:],
                    func=mybir.ActivationFunctionType.Relu)
                h2b = psm.tile([128, 128], dt.float16, name="h2b", tag="h2b")
                nc.vector.tensor_copy(out=h2b[:],
                                      in_=acc_b[:, b * 132:b * 132 + 128])
                nc.sync.dma_start(out=h2slice[b * 128:(b + 1) * 128, :],
                                  in_=h2b[:])

            def fin2(b):
                ssb = psm.tile([128, H], dt.float32, name="ssb2", tag="ssb")
                nc.vector.tensor_scalar_add(
                    out=ssb[:], in0=acc_a[:, b * 132 + 128:b * 132 + 132],
                    scalar1=1e-30)
                sr = psm.tile([128, H], dt.float32, name="sr2", tag="sr")
                nc.vector.reciprocal(out=sr[:], in_=ssb[:])
                rst = psm.tile([128, 128], dt.float32, name="rst2", tag="rst")
                nc.vector.tensor_tensor(
                    out=rst[:].rearrange("p (h d) -> p h d", h=H),
                    in0=acc_a[:, b * 132:b * 132 + 128].rearrange(
                        "p (h d) -> p h d", h=H),
                    in1=sr[:, :, None].to_broadcast([128, H, DH]),
                    op=mult)
                ob = psm.tile([128, 128], dt.float32, name="ob", tag="hb")
                nc.vector.tensor_tensor(out=ob[:], in0=rst[:], in1=b1_sb[:],
                                        op=add)
                nc.sync.dma_start(out=out_t[b * 128:(b + 1) * 128, :],
                                  in_=ob[:])

            # ---------------- layer 1 ----------------
            agg_pass(h0tab, acc_a, init_from_infeat=True,
                     after_block=(lambda b: feat_block(acc_a, 0, b))
                     if stages >= 2 else None)
            if stages >= 2:
                feat_tail(0)
            if stages >= 3:
                gat_pass(0, acc_a, after_block=fin1)
                maybe_cc(h2slice.opt(), h2full.opt())

            # ---------------- layer 2 ----------------
            if stages >= 4:
                agg_pass(h2full, acc_b, init_from_infeat=False,
                         after_block=lambda b: feat_block(acc_b, 1, b))
                feat_tail(1)
            if stages >= 5:
                gat_pass(1, acc_a, after_block=fin2)
            if stages < 5:
                src_acc = acc_a if stages < 4 else acc_b
                for b in range(NBLK):
                    nc.sync.dma_start(
                        out=out_t[b * 128:(b + 1) * 128, :],
                        in_=src_acc[:, b * 132:b * 132 + 128])

    nc.compile()
    return nc


def _compute_T(cfg_n, ncores, src, dst):
    NL = cfg_n // ncores
    NBLK = -(-NL // 128)
    c = dst // NL
    dl = dst - c * NL
    b = dl // 128
    g = (src >= cfg_n // 2).astype(np.int64)
    cell = (c * NBLK + b) * 2 + g
    cnt = np.bincount(cell, minlength=ncores * NBLK * 2)
    return int(-(-cnt.max() // 128))


def run_gat(n_nodes, ncores, CB, in_feat, edge_weights, W0, al0, ar0, b0,
            W1, al1, ar1, b1, src, dst, trace=False):
    """Shared entry: build, run on hardware, return full output."""
    T = _compute_T(n_nodes, ncores, src, dst)
    cfg = Cfg(n_nodes, ncores, T, CB)
    per_core, h0tab = _prep_host(cfg, in_feat, edge_weights, src, dst, b0)
    nc = _build(cfg)

    iota = np.tile(np.arange(128, dtype=F32)[None, :], (128, 1)).astype(
        np.float16)
    ident = np.eye(128, dtype=F32)
    shared = dict(
        h0tab=h0tab, w0=W0.astype(F32), w1=W1.astype(F32),
        alar0=_alar(al0, ar0), alar1=_alar(al1, ar1),
        b1t=np.tile(b1[None, :], (128, 1)).astype(F32),
        iota=np.ascontiguousarray(iota), ident=ident,
    )
    in_maps = []
    for ci in range(ncores):
        m = dict(shared)
        m.update(per_core[ci])
        in_maps.append(m)
    res = run_bass_kernel_spmd(nc, in_maps, core_ids=list(range(ncores)),
                               trace=trace)
    out = np.concatenate(
        [res.results[ci]["out"][:cfg.NL] for ci in range(ncores)], axis=0)
    return np.ascontiguousarray(out.astype(np.float32)), res


def kernel(**inputs):
    inputs = {k: np.asarray(v) for k, v in inputs.items()}
    out, _ = run_gat(
        N_NODES, NCORES, 7,
        inputs["in_feat"].astype(F32), inputs["edge_weights"].astype(F32),
        inputs["W0"], inputs["al0"], inputs["ar0"], inputs["b0"],
        inputs["W1"], inputs["al1"], inputs["ar1"], inputs["b1"],
        inputs["src"].astype(np.int64), inputs["dst"].astype(np.int64))
    return out
